# revision 5
# baseline (speedup 1.0000x reference)
"""Bilinear distance kernel for Trainium2 (8 NeuronCores, SPMD).

dists[b,n,m] = sum_{i,j} data[b,n,i] * W[0,i,j] * crit[b,m,j]
B=16, N=M=2048, LD=RD=128, fp32.

Sharding: data-parallel over B (2 batches per core). Per batch:
  dataT[i,n] , critT[j,m]  via PE transposes (contraction dim -> partitions)
  lwT[j,n]  = W.T @ dataT          (GEMM1, W stationary)
  out[n,m]  = lwT_tile.T @ critT   (GEMM2, fp32r full-rate)
Output writes (32 MiB/core) are the memory roofline.
"""

import sys

if "/opt/trn_rl_repo" not in sys.path:
    sys.path.insert(0, "/opt/trn_rl_repo")

import numpy as np

B, N, M, D = 16, 2048, 2048, 128
NCORES = 8
BPC = B // NCORES  # batches per core

_cache = {}


def _build():
    if "nc" in _cache:
        return _cache["nc"]

    import concourse.bacc as bacc
    import concourse.mybir as mybir
    from concourse import tile

    f32 = mybir.dt.float32
    f32r = mybir.dt.float32r

    nc = bacc.Bacc()
    data_d = nc.dram_tensor("data", [BPC, N, D], f32, kind="ExternalInput")
    crit_d = nc.dram_tensor("crit", [BPC, M, D], f32, kind="ExternalInput")
    w_d = nc.dram_tensor("w", [D, D], f32, kind="ExternalInput")
    out_d = nc.dram_tensor("out", [BPC, N, M], f32, kind="ExternalOutput")
    ident_d = nc.inline_tensor(np.eye(D, dtype=np.float32), name="ident")

    NT = N // 128          # 16 n-tiles per batch
    LG = 8                 # row-groups per load DMA (1 MiB loads)
    NL = N // (128 * LG)   # 2 load DMAs per tensor per batch
    SG = 4                 # n-tiles per store DMA (4 MiB stores)

    # engine rotation for GEMM2 psum->sbuf copies (2/3 DVE, 1/3 ACT)
    cp_k = 0

    with tile.TileContext(nc) as tc:
        with (
            tc.tile_pool(name="const", bufs=1) as cpool,
            tc.tile_pool(name="loads", bufs=2) as lpool,
            tc.tile_pool(name="big", bufs=2) as bigpool,
            tc.tile_pool(name="outs", bufs=2) as opool,
            tc.tile_pool(name="pst", bufs=3, space="PSUM") as pst,
            tc.tile_pool(name="psg", bufs=1, space="PSUM") as psg,
            tc.tile_pool(name="ps2", bufs=2, space="PSUM") as ps2,
        ):
            w_raw = cpool.tile([D, D], f32)
            nc.sync.dma_start(w_raw[:], w_d[:])
            w_sb = cpool.tile([D, D], f32r)
            nc.vector.tensor_copy(w_sb[:], w_raw[:])
            ident = cpool.tile([D, D], f32)
            nc.sync.dma_start(ident[:], ident_d[:])

            for b in range(BPC):
                dataT = bigpool.tile([D, N], f32r, tag="dataT")
                critT = bigpool.tile([D, M], f32r, tag="critT")
                lwT = bigpool.tile([D, N], f32r, tag="lwT")

                # ---- load + transpose data and crit (contraction -> partitions)
                for src_d, dstT, tg in ((data_d, dataT, "dl"), (crit_d, critT, "cl")):
                    for l in range(NL):
                        ld = lpool.tile([128, LG, D], f32, tag=tg)
                        nc.gpsimd.dma_start(
                            ld[:],
                            src_d[b, l * LG * 128 : (l + 1) * LG * 128, :].rearrange(
                                "(g p) d -> p g d", p=128
                            ),
                        )
                        for g in range(LG):
                            ps = pst.tile([128, 128], f32)
                            nc.tensor.transpose(ps[:], ld[:, g, :], ident[:])
                            t = l * LG + g
                            nc.vector.tensor_copy(
                                dstT[:, t * 128 : (t + 1) * 128], ps[:]
                            )

                # ---- GEMM1: lwT[j, n] = W.T @ dataT
                for c in range(N // 512):
                    ps = psg.tile([128, 512], f32)
                    nc.tensor.matmul(
                        ps[:],
                        w_sb[:],
                        dataT[:, c * 512 : (c + 1) * 512],
                        start=True,
                        stop=True,
                    )
                    nc.vector.tensor_copy(lwT[:, c * 512 : (c + 1) * 512], ps[:])

                # ---- GEMM2: out[n0:n0+128, :] = lwT_tile.T @ critT
                for sg in range(NT // SG):
                    ot = opool.tile([128, SG, M], f32, tag="ot")
                    for ntl in range(SG):
                        nt = sg * SG + ntl
                        lhs = lwT[:, nt * 128 : (nt + 1) * 128]
                        for h in range(2):  # two [128,1024] halves per n-tile
                            p2 = ps2.tile([128, 1024], f32)
                            for q in range(2):  # two 512-col matmuls per half
                                mc = h * 1024 + q * 512
                                nc.tensor.matmul(
                                    p2[:, q * 512 : (q + 1) * 512],
                                    lhs,
                                    critT[:, mc : mc + 512],
                                    start=True,
                                    stop=True,
                                )
                            eng = nc.vector if cp_k % 3 != 1 else nc.scalar
                            cp_k += 1
                            if eng is nc.vector:
                                eng.tensor_copy(
                                    ot[:, ntl, h * 1024 : (h + 1) * 1024], p2[:]
                                )
                            else:
                                eng.copy(ot[:, ntl, h * 1024 : (h + 1) * 1024], p2[:])
                    # 4 MiB contiguous store: rows sg*SG*128 .. (sg+1)*SG*128
                    st_eng = nc.sync if sg % 2 == 0 else nc.scalar
                    st_eng.dma_start(
                        out_d[
                            b, sg * SG * 128 : (sg + 1) * SG * 128, :
                        ].rearrange("(g p) m -> p g m", p=128),
                        ot[:],
                    )

    nc.finalize()
    _cache["nc"] = nc
    return nc


def kernel(data: np.ndarray, crit: np.ndarray, W: np.ndarray) -> np.ndarray:
    from concourse.bass_utils import run_bass_kernel_spmd

    nc = _build()
    data = np.ascontiguousarray(data, dtype=np.float32)
    crit = np.ascontiguousarray(crit, dtype=np.float32)
    w = np.ascontiguousarray(W.reshape(D, D), dtype=np.float32)
    in_maps = [
        {
            "data": data[c * BPC : (c + 1) * BPC],
            "crit": crit[c * BPC : (c + 1) * BPC],
            "w": w,
        }
        for c in range(NCORES)
    ]
    res = run_bass_kernel_spmd(nc, in_maps, core_ids=list(range(NCORES)))
    return np.concatenate([r["out"] for r in res.results], axis=0)


# revision 8
# speedup vs baseline: 1.0565x; 1.0565x over previous
"""Bilinear distance kernel for Trainium2 (8 NeuronCores, SPMD).

dists[b,n,m] = sum_{i,j} data[b,n,i] * W[0,i,j] * crit[b,m,j]
B=16, N=M=2048, LD=RD=128, fp32.

Sharding: data-parallel over B (2 batches per core). Per batch:
  dataT[i,n] , critT[j,m]  via PE transposes (contraction dim -> partitions)
  lwT[j,n]  = W.T @ dataT          (GEMM1, W stationary)
  out[n,m]  = lwT_tile.T @ critT   (GEMM2, fp32r full-rate)
Output writes (32 MiB/core) are the memory roofline.
"""

import sys

if "/opt/trn_rl_repo" not in sys.path:
    sys.path.insert(0, "/opt/trn_rl_repo")

import numpy as np

B, N, M, D = 16, 2048, 2048, 128
NCORES = 8
BPC = B // NCORES  # batches per core

_cache = {}


def _build():
    if "nc" in _cache:
        return _cache["nc"]

    import concourse.bacc as bacc
    import concourse.mybir as mybir
    from concourse import tile

    f32 = mybir.dt.float32
    f32r = mybir.dt.float32r

    nc = bacc.Bacc()
    data_d = nc.dram_tensor("data", [BPC, N, D], f32, kind="ExternalInput")
    crit_d = nc.dram_tensor("crit", [BPC, M, D], f32, kind="ExternalInput")
    w_d = nc.dram_tensor("w", [D, D], f32, kind="ExternalInput")
    out_d = nc.dram_tensor("out", [BPC, N, M], f32, kind="ExternalOutput")
    ident_d = nc.inline_tensor(np.eye(D, dtype=np.float32), name="ident")

    NT = N // 128          # 16 n-tiles per batch
    LG = 8                 # row-groups per load DMA (1 MiB loads)
    NL = N // (128 * LG)   # 2 load DMAs per tensor per batch
    SG = 2                 # n-tiles per store DMA (2 MiB stores)

    # engine rotation for GEMM2 psum->sbuf copies (2/3 DVE, 1/3 ACT)
    cp_k = 0

    with tile.TileContext(nc) as tc:
        with (
            tc.tile_pool(name="const", bufs=1) as cpool,
            tc.tile_pool(name="loads", bufs=2) as lpool,
            tc.tile_pool(name="big", bufs=2) as bigpool,
            tc.tile_pool(name="outs", bufs=3) as opool,
            tc.tile_pool(name="pst", bufs=3, space="PSUM") as pst,
            tc.tile_pool(name="psg", bufs=1, space="PSUM") as psg,
            tc.tile_pool(name="ps2", bufs=2, space="PSUM") as ps2,
        ):
            w_raw = cpool.tile([D, D], f32)
            nc.sync.dma_start(w_raw[:], w_d[:])
            w_sb = cpool.tile([D, D], f32r)
            nc.vector.tensor_copy(w_sb[:], w_raw[:])
            ident = cpool.tile([D, D], f32)
            nc.sync.dma_start(ident[:], ident_d[:])

            for b in range(BPC):
                dataT = bigpool.tile([D, N], f32r, tag="dataT")
                critT = bigpool.tile([D, M], f32r, tag="critT")
                lwT = bigpool.tile([D, N], f32r, tag="lwT")

                # ---- load + transpose crit then data (contraction -> partitions);
                # crit first: GEMM2 needs all of critT, shortening the fill.
                for src_d, dstT, tg, ldeng in (
                    (crit_d, critT, "cl", nc.sync),
                    (data_d, dataT, "dl", nc.scalar),
                ):
                    for l in range(NL):
                        ld = lpool.tile([128, LG, D], f32, tag=tg)
                        ldeng.dma_start(
                            ld[:],
                            src_d[b, l * LG * 128 : (l + 1) * LG * 128, :].rearrange(
                                "(g p) d -> p g d", p=128
                            ),
                        )
                        for g in range(LG):
                            ps = pst.tile([128, 128], f32)
                            nc.tensor.transpose(ps[:], ld[:, g, :], ident[:])
                            t = l * LG + g
                            nc.vector.tensor_copy(
                                dstT[:, t * 128 : (t + 1) * 128], ps[:]
                            )

                # ---- GEMM1: lwT[j, n] = W.T @ dataT
                for c in range(N // 512):
                    ps = psg.tile([128, 512], f32)
                    nc.tensor.matmul(
                        ps[:],
                        w_sb[:],
                        dataT[:, c * 512 : (c + 1) * 512],
                        start=True,
                        stop=True,
                    )
                    nc.vector.tensor_copy(lwT[:, c * 512 : (c + 1) * 512], ps[:])

                # ---- GEMM2: out[n0:n0+128, :] = lwT_tile.T @ critT
                for sg in range(NT // SG):
                    ot = opool.tile([128, SG, M], f32, tag="ot")
                    for ntl in range(SG):
                        nt = sg * SG + ntl
                        lhs = lwT[:, nt * 128 : (nt + 1) * 128]
                        for h in range(2):  # two [128,1024] halves per n-tile
                            p2 = ps2.tile([128, 1024], f32)
                            for q in range(2):  # two 512-col matmuls per half
                                mc = h * 1024 + q * 512
                                nc.tensor.matmul(
                                    p2[:, q * 512 : (q + 1) * 512],
                                    lhs,
                                    critT[:, mc : mc + 512],
                                    start=True,
                                    stop=True,
                                )
                            eng = nc.vector if cp_k % 3 != 1 else nc.scalar
                            cp_k += 1
                            if eng is nc.vector:
                                eng.tensor_copy(
                                    ot[:, ntl, h * 1024 : (h + 1) * 1024], p2[:]
                                )
                            else:
                                eng.copy(ot[:, ntl, h * 1024 : (h + 1) * 1024], p2[:])
                    # 4 MiB contiguous store: rows sg*SG*128 .. (sg+1)*SG*128
                    st_eng = nc.sync if sg % 2 == 0 else nc.scalar
                    st_eng.dma_start(
                        out_d[
                            b, sg * SG * 128 : (sg + 1) * SG * 128, :
                        ].rearrange("(g p) m -> p g m", p=128),
                        ot[:],
                    )

    nc.finalize()
    _cache["nc"] = nc
    return nc


def kernel(data: np.ndarray, crit: np.ndarray, W: np.ndarray) -> np.ndarray:
    from concourse.bass_utils import run_bass_kernel_spmd

    nc = _build()
    data = np.ascontiguousarray(data, dtype=np.float32)
    crit = np.ascontiguousarray(crit, dtype=np.float32)
    w = np.ascontiguousarray(W.reshape(D, D), dtype=np.float32)
    in_maps = [
        {
            "data": data[c * BPC : (c + 1) * BPC],
            "crit": crit[c * BPC : (c + 1) * BPC],
            "w": w,
        }
        for c in range(NCORES)
    ]
    res = run_bass_kernel_spmd(nc, in_maps, core_ids=list(range(NCORES)))
    return np.concatenate([r["out"] for r in res.results], axis=0)
